# revision 1
# baseline (speedup 1.0000x reference)
"""Causal multi-head self-attention on 8 TRN2 NeuronCores.

Sharding: batch (4) x head-group (2) -> 8 cores. Each core computes, for its
batch b and its 8 heads, the attention output projected through its slice of
Wo; the host sums the two partial outputs per batch.

Per-core layout (P = 128 partitions):
  XT   [128, 4, 8, 512] bf16 - x[b].T swizzled (quarter, d-chunk, col),
                               host pre-swizzled so each input quarter is one
                               contiguous 1MB DMA (descriptor-issue bound
                               startup: ~650ns per DMA instruction on the
                               HWDGE queues).
  WV   [128, 8, 512]  bf16 - Wv slice, host-swizzled, one DMA
  wsl  [128, 1024]    bf16 - per-pair Wq/Wk slices, host-swizzled, one DMA
  WO   [128, 4, 1024] f32r - Wo slice, host-swizzled, one DMA
  QT/KT [128, 2048] bf16 - head-dim on partitions (pair p -> tile p)
  V    16 tiles [128, 520] bf16 - seq on partitions, per-head 65-col groups
                                  (64 V cols + a ones col for row sums)
  scores computed transposed: S.T[k, q] = K @ Q.T, per head-pair via
  row-group packing (head A rows 0-63, head B rows 64-127).
  exp on ACT with fused 1/sqrt(dk) scale; causal via window-trimmed matmuls
  plus one [128,128] lower-triangle mask multiply on GpSimd (its queue is
  empty, keeping the exp->mask->AV chain off the vector-engine backlog).
  AV: O.T[65, q] += V_aug.T @ E.T accumulated over k-tiles in PSUM
  (row 0 = softmax denominators, ones col first). Normalize via vector
  reciprocal + PE broadcast (ones selector matmul). Final projection in
  bf16 (f32r would lower to the 2-pass fp32_mode=HIGH matmul path).
  Input DMAs are issued from both HWDGE queues (sync + scalar) to halve
  descriptor-issue latency at startup.
"""

import numpy as np
import ml_dtypes

import concourse.bass as bass
import concourse.tile as tile
from concourse import bacc, mybir
from concourse import bass_utils

F32 = mybir.dt.float32
F32R = mybir.dt.float32r
BF16 = mybir.dt.bfloat16
NPBF16 = ml_dtypes.bfloat16

B, S, D, H, DK = 4, 2048, 1024, 16, 64
HC = 8          # heads per core
NPAIR = 4       # head pairs per core
OC = 512        # output dims per core (= HC * DK)
KT_N = 16       # seq k-tiles of 128
SCH = 4         # seq chunks of 512
SCALE = 1.0 / np.sqrt(np.float32(DK))

_CACHE = {}


def _emit(nc, tc, dram):
    P = 128
    xsw_d, wqsw_d, wksw_d, wvsw_d, wosw_d, trimask_d, sel_d, out_d = (
        dram["xsw"], dram["wqsw"], dram["wksw"], dram["wvsw"], dram["wosw"],
        dram["trimask"], dram["sel2"], dram["out"],
    )

    import contextlib
    ctx = contextlib.ExitStack()
    with ctx:
        # ---------------- persistent SBUF ----------------
        per = ctx.enter_context(tc.tile_pool(name="per", bufs=1))
        trimask = per.tile([P, P], BF16, tag="trimask", name="trimask")
        sel2 = per.tile([2, P], BF16, tag="sel2", name="sel2")

        QT = [per.tile([P, S], BF16, tag=f"QT{p}", name=f"QT{p}") for p in range(NPAIR)]
        KT = [per.tile([P, S], BF16, tag=f"KT{p}", name=f"KT{p}") for p in range(NPAIR)]
        V = [per.tile([P, HC * 65], BF16, tag=f"V{t}", name=f"V{t}") for t in range(KT_N)]
        # OT / WO / stage are bf16: f32r operands lower to the 2-pass
        # fp32_mode=HIGH matmul path (~3x slower, LDWEIGHTS not overlapped)
        OT = [per.tile([P, S], BF16, tag=f"OT{p}", name=f"OT{p}") for p in range(NPAIR)]
        WO = per.tile([P, NPAIR, D], BF16, tag="WO", name="WO")

        epool = ctx.enter_context(tc.tile_pool(name="epool", bufs=6))
        stg = ctx.enter_context(tc.tile_pool(name="stg", bufs=1))

        with tc.tile_pool(name="proj_in", bufs=1) as proj_in, \
             tc.tile_pool(name="wqk", bufs=2) as wqk_pool, \
             tc.tile_pool(name="ps", bufs=1, space="PSUM") as psp:
            XT = proj_in.tile([P, SCH, 8, 512], BF16, tag="XT", name="XT")
            WV = proj_in.tile([P, 8, 512], BF16, tag="WV", name="WV")

            # PSUM budget (8 banks of [128,512]f32):
            #   pp: 2 x [128,512]  = 2 banks (projection accumulators)
            #   s:  2 x [128,1024] = 4 banks (scores, both heads)
            #   o:  1 x [128,1024] = 2 banks (output accum, both heads)
            def pp_tile():
                return psp.tile([P, OC], F32, tag="pp", bufs=2, name="pp")

            def fill(it, n):
                for _ in range(n):
                    fn = next(it, None)
                    if fn is None:
                        return
                    fn()

            # ----- V projection: V[st] = x[st*128:+128, :] @ WvT, seq on
            # partitions. Emitted in quarter groups so it can interleave with
            # pair-0 QK projections while input quarters are still landing.
            def v_group(g):
                for st in range(4 * g, 4 * g + 4):
                    qq, bb = divmod(st, 4)
                    ps = pp_tile()
                    for k in range(8):
                        nc.tensor.matmul(
                            ps[:], XT[:, qq, k, bb * P:(bb + 1) * P], WV[:, k],
                            start=(k == 0), stop=(k == 7),
                        )
                    # ones column FIRST: the AV sums row lands on o2 row 0
                    v3 = V[st].rearrange("p (h d) -> p h d", d=65)
                    nc.vector.tensor_copy(
                        v3[:, :, 1:65], ps[:].rearrange("p (h d) -> p h d", d=64)
                    )
                    nc.gpsimd.memset(v3[:, :, 0:1], 1.0)

            # ----- QK projections + attention, pair by pair.
            # The PE executes its stream in order and the attention j-loop is
            # ACT(exp)-bound, so projection / final-projection matmuls are
            # interleaved as per-MM filler units inside the j-loop.
            def proj_units(p, eng=None):
                # yields: 2 weight-load units, then per sc the q-proj and
                # k-proj of that chunk (so startup can gate on XT quarters)
                units = []
                load_units = []
                boxes = []
                for (wd,) in ((wqsw_d,), (wksw_d,)):
                    def load_w(wd=wd):
                        wsl = wqk_pool.tile([P, 8 * P], BF16, tag="wsl",
                                            name="wsl")
                        (eng or nc.sync).dma_start(
                            wsl[:], wd[p * P:(p + 1) * P, :])
                        return wsl
                    wsl_box = []
                    boxes.append(wsl_box)
                    load_units.append(lambda wsl_box=wsl_box, load_w=load_w:
                                      wsl_box.append(load_w()))
                for sc in range(SCH):
                    for wi, dst in ((0, QT[p]), (1, KT[p])):
                        ps_box = []
                        wsl_box = boxes[wi]
                        for k in range(8):
                            def mm(k=k, sc=sc, ps_box=ps_box,
                                   wsl_box=wsl_box):
                                if k == 0:
                                    ps_box.append(pp_tile())
                                nc.tensor.matmul(
                                    ps_box[0][:],
                                    wsl_box[0][:, k * P:(k + 1) * P],
                                    XT[:, sc, k],
                                    start=(k == 0), stop=(k == 7),
                                )
                            units.append(mm)
                        def cp(sc=sc, ps_box=ps_box, dst=dst):
                            nc.vector.tensor_copy(
                                dst[:, sc * 512:(sc + 1) * 512], ps_box[0][:])
                        units.append(cp)
                # both weight-slice DMAs lead the unit stream so neither
                # projection's first matmul waits on its load
                return iter(load_units + units)

            def final_units(cc, pre=None, dma_eng=None):
                # pre: optional per-t hook run before t's matmuls (tail norm)
                # dma_eng: queue for the output DMA (tail uses scalar: ACT is
                # idle there and same-queue issue skips a semaphore hop)
                units = []
                for t in range(4 * cc, 4 * cc + 4):
                    if pre is not None:
                        units.append(pre(t))
                    ostg_box = []
                    for oc in range(2):
                        ps_box = []
                        for p4 in range(NPAIR):
                            def mm(p4=p4, t=t, oc=oc, ps_box=ps_box):
                                if p4 == 0:
                                    ps_box.append(
                                        psp.tile([P, OC], F32, tag="pp",
                                                 bufs=2, name="pp"))
                                nc.tensor.matmul(
                                    ps_box[0][:],
                                    OT[p4][:, t * P:(t + 1) * P],
                                    WO[:, p4, oc * 512:(oc + 1) * 512],
                                    start=(p4 == 0), stop=(p4 == NPAIR - 1),
                                )
                            units.append(mm)
                        def cp(t=t, oc=oc, ps_box=ps_box, ostg_box=ostg_box):
                            if oc == 0:
                                ostg_box.append(
                                    stg.tile([P, 2 * OC], BF16, tag="ostg",
                                             bufs=3, name="ostg"))
                            # as a pair-3 attention filler (dma_eng None)
                            # this copy must NOT sit on ACT: each one pushes
                            # the exp chain (the attention pacer) back
                            # ~0.7us. DVE is light there (no proj CASTs).
                            cpeng = nc.scalar if dma_eng is not None \
                                else nc.vector
                            if cpeng is nc.scalar:
                                nc.scalar.copy(
                                    ostg_box[0][:, oc * 512:(oc + 1) * 512],
                                    ps_box[0][:])
                            else:
                                nc.vector.tensor_copy(
                                    ostg_box[0][:, oc * 512:(oc + 1) * 512],
                                    ps_box[0][:])
                            if dma_eng is not None:
                                # tail: DMA each half right after its copy
                                # so transfer overlaps the other half's copy
                                dma_eng.dma_start(
                                    out_d[t * P:(t + 1) * P,
                                          oc * 512:(oc + 1) * 512],
                                    ostg_box[0][:, oc * 512:(oc + 1) * 512])
                            elif oc == 1:
                                nc.sync.dma_start(
                                    out_d[t * P:(t + 1) * P, :], ostg_box[0][:])
                        units.append(cp)
                return iter(units)

            # ----- startup: early DMA bandwidth is ~215GB/s TOTAL no matter
            # how many queues are used (they contend), so all inputs go on
            # the scalar queue (issues ~2us earlier than sync) in exact
            # first-use order. Compute is emitted in the same order so the
            # PE runs dense from the first arrival.
            pu0 = proj_units(0, eng=nc.scalar)
            fill(pu0, 2)  # the two pair-0 weight-slice DMAs, on scalar
            xsw3 = xsw_d.rearrange("p (q k c) -> p q k c", q=SCH, k=8)
            # quarter 0 split in two k-halves: the first four contraction
            # matmuls gate on 512KB instead of 1MB (~2us earlier start)
            nc.scalar.dma_start(XT[:, 0, 0:4], xsw3[:, 0, 0:4])
            nc.scalar.dma_start(XT[:, 0, 4:8], xsw3[:, 0, 4:8])
            nc.scalar.dma_start(WV[:], wvsw_d.rearrange("p (k c) -> p k c", k=8))
            nc.scalar.dma_start(XT[:, 1], xsw3[:, 1])
            nc.scalar.dma_start(XT[:, 2], xsw3[:, 2])
            nc.scalar.dma_start(XT[:, 3], xsw3[:, 3])
            nc.scalar.dma_start(trimask[:], trimask_d[:, :])
            nc.scalar.dma_start(sel2[:], sel_d[:, :])

            norm_q = []
            stage_q = []
            # wsl+q0 -> sc0, +WV -> V group 0, +q1 -> sc1, V1, ...
            fill(pu0, 18)
            v_group(0)
            fill(pu0, 18)
            v_group(1)
            fill(pu0, 18)
            v_group(2)
            fill(pu0, 18)
            v_group(3)
            fill(pu0, 10 ** 6)
            def mk_filln(p):
                # pairs 0-2 have 74 filler units for 80 slots; place the
                # shortfall at c2's start (79% PE density) instead of the
                # pair's last slots (60% density -> HAM re-throttle)
                if p < NPAIR - 1:
                    return lambda c, j: 1 if (c == 2 and j < 6) else 2
                return lambda c, j: 2

            for p in range(NPAIR):
                if p < NPAIR - 1:
                    pu = proj_units(p + 1)
                    fillers = [pu, pu, pu, pu]
                else:
                    nc.sync.dma_start(
                        WO[:], wosw_d.rearrange("p (f d) -> p f d", f=NPAIR))
                    fillers = [iter(()), final_units(0), final_units(1),
                               final_units(2)]
                _attention_pair(nc, tc, psp, epool, stg, p, QT, KT, V, OT,
                                sel2, trimask, norm_q, stage_q, fillers,
                                mk_filln(p), True)
            # tail: pair-3 chunk-3 staging, then its norm split per t-block
            # and interleaved with the final projection of blocks 12..15
            stage_q.pop(0)()
            assert len(norm_q) == 1
            bc_box = []
            norm_bc, norm_mul_t = norm_q.pop(0)

            def pre(t):
                def fn(t=t):
                    if not bc_box:
                        bc_box.append(norm_bc(long_lived=True))
                    norm_mul_t(bc_box[0], t, rcp=True)
                return fn
            for fn in final_units(3, pre=pre, dma_eng=nc.scalar):
                fn()


def _attention_pair(nc, tc, psp, epool, stg, p, QT, KT, V, OT, sel2,
                    trimask, norm_q, stage_q, fillers, fill_n, drain_last):
    P = 128

    def fill(it, n):
        for _ in range(n):
            fn = next(it, None)
            if fn is None:
                return
            fn()

    last_pair = (p == NPAIR - 1)
    for c in range(SCH):
        filler = fillers[c]
        pend_stage = stage_q.pop(0) if stage_q else None
        pend_norm = norm_q.pop(0) if norm_q else None
        if last_pair:
            # pair 3's fillers (final projection) read OT at the first fill
            # slots, so staging+norm cannot be deferred into the j-loop
            if pend_stage is not None:
                pend_stage()
                pend_stage = None
            if pend_norm is not None:
                norm_bc, norm_mul_t = pend_norm
                norm_mul_t(norm_bc(), None)
                pend_norm = None
        o2 = psp.tile([P, 1024], F32, tag="o", bufs=1, name="o2")
        njt = 4 * c + 4
        pend_av = []
        for j in range(njt):
            d = j - 4 * c
            w = d * P if d >= 0 else 0
            s2 = psp.tile([P, 1024], F32, tag="s", bufs=2, name="s2")
            for hh in range(2):
                nc.tensor.matmul(
                    s2[:, hh * 512 + w: hh * 512 + 512],
                    KT[p][hh * 64:(hh + 1) * 64, j * P:(j + 1) * P],
                    QT[p][hh * 64:(hh + 1) * 64, c * 512 + w:(c + 1) * 512],
                    start=True, stop=True,
                )
            e2 = epool.tile([P, 1024], BF16, tag="e", name="e2")
            nc.scalar.activation(
                e2[:].rearrange("p (h q) -> p h q", h=2)[:, :, w:512],
                s2[:].rearrange("p (h q) -> p h q", h=2)[:, :, w:512],
                mybir.ActivationFunctionType.Exp,
                scale=float(SCALE),
            )
            if j == 0 and pend_stage is not None:
                # previous chunk's staging, emitted AFTER this chunk's first
                # exp: the stage copy waits on the last AVs, and putting it
                # first in the ACT queue would block the new chunk's exps
                # (head-of-line) and dip PE density enough to re-throttle
                # the HAM clock for ~10us
                pend_stage()
                pend_stage = None
            if j == (3 if c == 0 else 2) and pend_norm is not None:
                # chunk 0: pop one slot later -- the bc matmul's sums-hop
                # dependency (stage->sb2 DMA, ~2.3us after the chunk start)
                # races a j==2 pop (~2.2us in) at pair crossings
                norm_bc, norm_mul_t = pend_norm
                norm_mul_t(norm_bc(), None)
                pend_norm = None
            if d >= 0:
                # causal mask of the diagonal block, alternating WHOLE j's
                # between GpSimd and Vector: each j has its own e2 tile, so
                # this parallelizes the engines without the same-tile
                # cross-engine serialization a per-head split causes.
                eng = nc.gpsimd if (j % 2 == 0) else nc.vector
                for hh in range(2):
                    blk = e2[:, hh * 512 + w: hh * 512 + w + P]
                    eng.tensor_mul(blk, blk, trimask[:])
            fill(filler, fill_n(c, j))
            # chunk 0 is short and all-diagonal: defer AVs one slot deeper
            # so they clear the previous chunk's staging WAR and the mask
            # multiplies without stalling the PE
            if len(pend_av) == (3 if c == 0 else 2):
                pend_av.pop(0)()

            def av(j=j, w=w, e2=e2, o2=o2, njt=njt):
                for hh in range(2):
                    head = 2 * p + hh
                    nc.tensor.matmul(
                        o2[0:65, hh * 512 + w: hh * 512 + 512],
                        V[j][:, head * 65: head * 65 + 65],
                        e2[:, hh * 512 + w: hh * 512 + 512],
                        start=(j == 0), stop=(j == njt - 1),
                    )
            pend_av.append(av)
        for fn in pend_av:
            fn()
        # PSUM -> SBUF staging (engines cannot shift partitions; DMA cannot
        # read PSUM), then SBUF->SBUF DMAs to place head A / B rows. Row 0
        # of stage holds both heads' softmax denominators. Deferred into the
        # next chunk's j-loop (j==0) via stage_q.
        sb2_box = []

        def _staging(o2=o2, sb2_box=sb2_box, p=p, c=c):
            stage = stg.tile([65, 1024], BF16, tag="stage", bufs=2,
                             name="stage")
            # single whole-tile ACT copy (split variants measured worse),
            # sums in row 0, hop DMAs shift the body rows into OT
            nc.scalar.copy(stage[:], o2[0:65, :])
            nc.sync.dma_start(
                OT[p][0:64, c * 512:(c + 1) * 512], stage[1:65, 0:512])
            nc.sync.dma_start(
                OT[p][64:128, c * 512:(c + 1) * 512], stage[1:65, 512:1024])
            sb2 = stg.tile([2, 512], BF16, tag="sb2", bufs=3, name="sb2")
            nc.sync.dma_start(
                sb2[:, :], stage[0:1, :].rearrange("a (h c) -> a h c", h=2))
            sb2_box.append(sb2)
        stage_q.append(_staging)

        def _norm_bc(sb2_box=sb2_box, long_lived=False):
            sb2 = sb2_box[0]
            # Deferred into the next chunk's j-loop so the PE-stream
            # position of the bc matmul is past its dependencies.
            # Broadcast raw sums, then invert + multiply on DVE.
            # The tail norm (long_lived) must survive across final-unit
            # "pp" allocations, so it reuses the dead scores region; its
            # reciprocal runs on ACT (0.7us vs 3.4us on DVE) -- safe only
            # there because all exps are done, so no table-set thrash.
            if long_lived:
                bc = psp.tile([P, 1024], F32, tag="s", bufs=2,
                              name="s2")[:, 0:OC]
            else:
                bc = psp.tile([P, OC], F32, tag="pp", bufs=2, name="pp")
            # reciprocal lands in SBUF so the PSUM pp buffer is released
            # after ONE DVE op -- later filler matmuls reusing the pp tag
            # would otherwise stall on the whole norm chain
            rbc = stg.tile([P, OC], F32, tag="rbc", bufs=2, name="rbc")
            nc.tensor.matmul(bc[:], sel2[:], sb2[:], start=True, stop=True)
            if not long_lived:
                nc.vector.reciprocal(rbc[:], bc[:])
            return (bc, rbc)

        def _norm_mul_t(bcp, t, p=p, c=c, rcp=False):
            # t None: whole chunk; else one 128-col block of the chunk.
            # rcp: invert the block here (tail path: per-block reciprocal
            # pipelines with the per-t final projections instead of one
            # serial 3.4us reciprocal).
            bc, rbc = bcp
            if t is None:
                cols = slice(c * 512, (c + 1) * 512)
                bcols = slice(0, 512)
            else:
                tb = t - 4 * c
                cols = slice(c * 512 + tb * P, c * 512 + (tb + 1) * P)
                bcols = slice(tb * P, (tb + 1) * P)
            if rcp:
                nc.vector.reciprocal(rbc[:, bcols], bc[:, bcols])
            nc.vector.tensor_mul(
                OT[p][:, cols], OT[p][:, cols], rbc[:, bcols])
        norm_q.append((_norm_bc, _norm_mul_t))
        # drain leftover filler, but only when the next chunk doesn't
        # continue the same iterator (projection fillers span the pair;
        # pair 2 deliberately leaves 8 units for pair 3's chunk 0)
        if (c == SCH - 1 and drain_last) or \
                (c < SCH - 1 and fillers[c + 1] is not filler):
            fill(filler, 10 ** 6)


def _build():
    if "nc" in _CACHE:
        return _CACHE["nc"]
    nc = bacc.Bacc("TRN2", target_bir_lowering=False, debug=False)
    dram = {
        "xsw": nc.dram_tensor("xsw", [128, 16384], BF16,
                              kind="ExternalInput").ap(),
        "wqsw": nc.dram_tensor("wqsw", [512, 1024], BF16,
                               kind="ExternalInput").ap(),
        "wksw": nc.dram_tensor("wksw", [512, 1024], BF16,
                               kind="ExternalInput").ap(),
        "wvsw": nc.dram_tensor("wvsw", [128, 4096], BF16,
                               kind="ExternalInput").ap(),
        "wosw": nc.dram_tensor("wosw", [128, 4096], BF16,
                               kind="ExternalInput").ap(),
        "trimask": nc.dram_tensor("trimask", [128, 128], BF16,
                                  kind="ExternalInput").ap(),
        "sel2": nc.dram_tensor("sel2", [2, 128], BF16,
                               kind="ExternalInput").ap(),
        "out": nc.dram_tensor("out", [S, D], BF16, kind="ExternalOutput").ap(),
    }
    with tile.TileContext(nc) as tc:
        _emit(nc, tc, dram)
    nc.compile()
    _CACHE["nc"] = nc
    return nc


def make_in_maps(x, Wq, Wk, Wv, Wo):
    x = np.asarray(x, np.float32)
    Wq = np.asarray(Wq, np.float32)
    Wk = np.asarray(Wk, np.float32)
    Wv = np.asarray(Wv, np.float32)
    Wo = np.asarray(Wo, np.float32)
    tri = np.tril(np.ones((128, 128), np.float32)).T.astype(NPBF16)
    sel = np.zeros((2, 128), np.float32)
    sel[0, 0:64] = 1.0
    sel[1, 64:128] = 1.0
    in_maps = []
    for cidx in range(8):
        b, g = divmod(cidx, 2)
        sl = slice(g * OC, (g + 1) * OC)
        xT = np.ascontiguousarray(x[b].T)                   # [1024, 2048]
        # [p, q, k, c]: xT[k*128+p, q*512+c]
        xsw = np.ascontiguousarray(
            xT.reshape(8, 128, 4, 512).transpose(1, 2, 0, 3)
        ).reshape(128, 16384)
        WqT = Wq[sl, :].T                                   # [1024, 512]
        WkT = Wk[sl, :].T
        WvT = Wv[sl, :].T
        WoT = Wo[:, sl].T                                   # [512, 1024]
        # wsl layout per pair: [pair*128+p, k*128+c] = WqT[k*128+p, pair*128+c]
        wqsw = np.ascontiguousarray(
            WqT.reshape(8, 128, 4, 128).transpose(2, 1, 0, 3)
        ).reshape(512, 1024)
        wksw = np.ascontiguousarray(
            WkT.reshape(8, 128, 4, 128).transpose(2, 1, 0, 3)
        ).reshape(512, 1024)
        # [p, k*512+c] = WvT[k*128+p, c]
        wvsw = np.ascontiguousarray(
            WvT.reshape(8, 128, 512).transpose(1, 0, 2)
        ).reshape(128, 4096)
        # [p, p4*1024+d] = WoT[p4*128+p, d]
        wosw = np.ascontiguousarray(
            WoT.reshape(4, 128, 1024).transpose(1, 0, 2)
        ).reshape(128, 4096)
        in_maps.append({
            "xsw": xsw.astype(NPBF16),
            "wqsw": wqsw.astype(NPBF16),
            "wksw": wksw.astype(NPBF16),
            "wvsw": wvsw.astype(NPBF16),
            "wosw": wosw.astype(NPBF16),
            "trimask": tri,
            "sel2": sel.astype(NPBF16),
        })
    return in_maps


def combine(results):
    parts = [np.asarray(results[c]["out"]).astype(np.float32)
             for c in range(8)]
    return np.stack([parts[2 * b] + parts[2 * b + 1] for b in range(B)])


def kernel(**inputs):
    nc = _build()
    in_maps = make_in_maps(inputs["x"], inputs["Wq"], inputs["Wk"],
                           inputs["Wv"], inputs["Wo"])
    res = bass_utils.run_bass_kernel_spmd(nc, in_maps, core_ids=list(range(8)))
    return combine(res.results)


def run_traced(**inputs):
    nc = _build()
    in_maps = make_in_maps(inputs["x"], inputs["Wq"], inputs["Wk"],
                           inputs["Wv"], inputs["Wo"])
    res = bass_utils.run_bass_kernel_spmd(
        nc, in_maps, core_ids=list(range(8)), trace=True)
    return combine(res.results), res


def run_timed(**inputs):
    """One more traced execution, returning just the exec time (repeat
    measurements to average out the chip power-state noise)."""
    nc = _build()
    in_maps = make_in_maps(inputs["x"], inputs["Wq"], inputs["Wk"],
                           inputs["Wv"], inputs["Wo"])
    res = bass_utils.run_bass_kernel_spmd(
        nc, in_maps, core_ids=list(range(8)), trace=True)
    return res.exec_time_ns



# revision 5
# speedup vs baseline: 1.0674x; 1.0674x over previous
"""Causal multi-head self-attention on 8 TRN2 NeuronCores.

Sharding: batch (4) x head-group (2) -> 8 cores. Each core computes, for its
batch b and its 8 heads, the attention output projected through its slice of
Wo; the host sums the two partial outputs per batch.

Per-core layout (P = 128 partitions):
  XT   [128, 4, 8, 512] bf16 - x[b].T swizzled (quarter, d-chunk, col),
                               host pre-swizzled so each input quarter is one
                               contiguous 1MB DMA (descriptor-issue bound
                               startup: ~650ns per DMA instruction on the
                               HWDGE queues).
  WV   [128, 8, 512]  bf16 - Wv slice, host-swizzled, one DMA
  wsl  [128, 1024]    bf16 - per-pair Wq/Wk slices, host-swizzled, one DMA
  WO   [128, 4, 1024] f32r - Wo slice, host-swizzled, one DMA
  QT/KT [128, 2048] bf16 - head-dim on partitions (pair p -> tile p)
  V    16 tiles [128, 520] bf16 - seq on partitions, per-head 65-col groups
                                  (64 V cols + a ones col for row sums)
  scores computed transposed: S.T[k, q] = K @ Q.T, per head-pair via
  row-group packing (head A rows 0-63, head B rows 64-127).
  exp on ACT with fused 1/sqrt(dk) scale; causal via window-trimmed matmuls
  plus one [128,128] lower-triangle mask multiply on GpSimd (its queue is
  empty, keeping the exp->mask->AV chain off the vector-engine backlog).
  AV: O.T[65, q] += V_aug.T @ E.T accumulated over k-tiles in PSUM
  (row 0 = softmax denominators, ones col first). Normalize via vector
  reciprocal + PE broadcast (ones selector matmul). Final projection in
  bf16 (f32r would lower to the 2-pass fp32_mode=HIGH matmul path).
  Input DMAs are issued from both HWDGE queues (sync + scalar) to halve
  descriptor-issue latency at startup.
"""

import numpy as np
import ml_dtypes

import concourse.bass as bass
import concourse.tile as tile
from concourse import bacc, mybir
from concourse import bass_utils

F32 = mybir.dt.float32
F32R = mybir.dt.float32r
BF16 = mybir.dt.bfloat16
NPBF16 = ml_dtypes.bfloat16

B, S, D, H, DK = 4, 2048, 1024, 16, 64
HC = 8          # heads per core
NPAIR = 4       # head pairs per core
OC = 512        # output dims per core (= HC * DK)
KT_N = 16       # seq k-tiles of 128
SCH = 4         # seq chunks of 512
SCALE = 1.0 / np.sqrt(np.float32(DK))

_CACHE = {}


def _emit(nc, tc, dram):
    P = 128
    xsw_d, wqsw_d, wksw_d, wvsw_d, wosw_d, trimask_d, sel_d, out_d = (
        dram["xsw"], dram["wqsw"], dram["wksw"], dram["wvsw"], dram["wosw"],
        dram["trimask"], dram["sel2"], dram["out"],
    )

    import contextlib
    ctx = contextlib.ExitStack()
    with ctx:
        # ---------------- persistent SBUF ----------------
        per = ctx.enter_context(tc.tile_pool(name="per", bufs=1))
        trimask = per.tile([P, P], BF16, tag="trimask", name="trimask")
        sel2 = per.tile([2, P], BF16, tag="sel2", name="sel2")

        QT = [per.tile([P, S], BF16, tag=f"QT{p}", name=f"QT{p}") for p in range(NPAIR)]
        KT = [per.tile([P, S], BF16, tag=f"KT{p}", name=f"KT{p}") for p in range(NPAIR)]
        V = [per.tile([P, HC * 65], BF16, tag=f"V{t}", name=f"V{t}") for t in range(KT_N)]
        # OT / WO / stage are bf16: f32r operands lower to the 2-pass
        # fp32_mode=HIGH matmul path (~3x slower, LDWEIGHTS not overlapped)
        OT = [per.tile([P, S], BF16, tag=f"OT{p}", name=f"OT{p}") for p in range(NPAIR)]
        WO = per.tile([P, NPAIR, D], BF16, tag="WO", name="WO")

        epool = ctx.enter_context(tc.tile_pool(name="epool", bufs=6))
        stg = ctx.enter_context(tc.tile_pool(name="stg", bufs=1))

        with tc.tile_pool(name="proj_in", bufs=1) as proj_in, \
             tc.tile_pool(name="wqk", bufs=2) as wqk_pool, \
             tc.tile_pool(name="ps", bufs=1, space="PSUM") as psp:
            XT = proj_in.tile([P, SCH, 8, 512], BF16, tag="XT", name="XT")
            WV = proj_in.tile([P, 8, 512], BF16, tag="WV", name="WV")

            # PSUM budget (8 banks of [128,512]f32):
            #   pp: 2 x [128,512]  = 2 banks (projection accumulators)
            #   s:  2 x [128,1024] = 4 banks (scores, both heads)
            #   o:  1 x [128,1024] = 2 banks (output accum, both heads)
            def pp_tile():
                return psp.tile([P, OC], F32, tag="pp", bufs=2, name="pp")

            def fill(it, n):
                for _ in range(n):
                    fn = next(it, None)
                    if fn is None:
                        return
                    fn()

            # ----- V projection: V[st] = x[st*128:+128, :] @ WvT, seq on
            # partitions. Emitted in quarter groups so it can interleave with
            # pair-0 QK projections while input quarters are still landing.
            def v_group(g):
                for st in range(4 * g, 4 * g + 4):
                    qq, bb = divmod(st, 4)
                    ps = pp_tile()
                    for k in range(8):
                        nc.tensor.matmul(
                            ps[:], XT[:, qq, k, bb * P:(bb + 1) * P], WV[:, k],
                            start=(k == 0), stop=(k == 7),
                        )
                    # ones column FIRST: the AV sums row lands on o2 row 0
                    v3 = V[st].rearrange("p (h d) -> p h d", d=65)
                    nc.vector.tensor_copy(
                        v3[:, :, 1:65], ps[:].rearrange("p (h d) -> p h d", d=64)
                    )
                    nc.gpsimd.memset(v3[:, :, 0:1], 1.0)

            # ----- QK projections + attention, pair by pair.
            # The PE executes its stream in order and the attention j-loop is
            # ACT(exp)-bound, so projection / final-projection matmuls are
            # interleaved as per-MM filler units inside the j-loop.
            def proj_units(p, eng=None):
                # yields: 2 weight-load units, then per sc the q-proj and
                # k-proj of that chunk (so startup can gate on XT quarters)
                units = []
                load_units = []
                boxes = []
                for (wd,) in ((wqsw_d,), (wksw_d,)):
                    def load_w(wd=wd):
                        wsl = wqk_pool.tile([P, 8 * P], BF16, tag="wsl",
                                            name="wsl")
                        (eng or nc.sync).dma_start(
                            wsl[:], wd[p * P:(p + 1) * P, :])
                        return wsl
                    wsl_box = []
                    boxes.append(wsl_box)
                    load_units.append(lambda wsl_box=wsl_box, load_w=load_w:
                                      wsl_box.append(load_w()))
                for sc in range(SCH):
                    for wi, dst in ((0, QT[p]), (1, KT[p])):
                        ps_box = []
                        wsl_box = boxes[wi]
                        for k in range(8):
                            def mm(k=k, sc=sc, ps_box=ps_box,
                                   wsl_box=wsl_box):
                                if k == 0:
                                    ps_box.append(pp_tile())
                                nc.tensor.matmul(
                                    ps_box[0][:],
                                    wsl_box[0][:, k * P:(k + 1) * P],
                                    XT[:, sc, k],
                                    start=(k == 0), stop=(k == 7),
                                )
                            units.append(mm)
                        def cp(sc=sc, ps_box=ps_box, dst=dst):
                            nc.vector.tensor_copy(
                                dst[:, sc * 512:(sc + 1) * 512], ps_box[0][:])
                        units.append(cp)
                # both weight-slice DMAs lead the unit stream so neither
                # projection's first matmul waits on its load
                return iter(load_units + units)

            def final_units(cc, pre=None, dma_eng=None):
                # pre: optional per-t hook run before t's matmuls (tail norm)
                # dma_eng: queue for the output DMA (tail uses scalar: ACT is
                # idle there and same-queue issue skips a semaphore hop)
                units = []
                for t in range(4 * cc, 4 * cc + 4):
                    if pre is not None:
                        units.append(pre(t))
                    ostg_box = []
                    for oc in range(2):
                        ps_box = []
                        for p4 in range(NPAIR):
                            def mm(p4=p4, t=t, oc=oc, ps_box=ps_box):
                                if p4 == 0:
                                    ps_box.append(
                                        psp.tile([P, OC], F32, tag="pp",
                                                 bufs=2, name="pp"))
                                nc.tensor.matmul(
                                    ps_box[0][:],
                                    OT[p4][:, t * P:(t + 1) * P],
                                    WO[:, p4, oc * 512:(oc + 1) * 512],
                                    start=(p4 == 0), stop=(p4 == NPAIR - 1),
                                )
                            units.append(mm)
                        def cp(t=t, oc=oc, ps_box=ps_box, ostg_box=ostg_box):
                            if oc == 0:
                                ostg_box.append(
                                    stg.tile([P, 2 * OC], BF16, tag="ostg",
                                             bufs=3, name="ostg"))
                            # as a pair-3 attention filler (dma_eng None)
                            # this copy must NOT sit on ACT: each one pushes
                            # the exp chain (the attention pacer) back
                            # ~0.7us. DVE is light there (no proj CASTs).
                            cpeng = nc.scalar if dma_eng is not None \
                                else nc.vector
                            if cpeng is nc.scalar:
                                nc.scalar.copy(
                                    ostg_box[0][:, oc * 512:(oc + 1) * 512],
                                    ps_box[0][:])
                            else:
                                nc.vector.tensor_copy(
                                    ostg_box[0][:, oc * 512:(oc + 1) * 512],
                                    ps_box[0][:])
                            if dma_eng is not None:
                                # tail: DMA each half right after its copy
                                # so transfer overlaps the other half's copy
                                dma_eng.dma_start(
                                    out_d[t * P:(t + 1) * P,
                                          oc * 512:(oc + 1) * 512],
                                    ostg_box[0][:, oc * 512:(oc + 1) * 512])
                            elif oc == 1:
                                nc.sync.dma_start(
                                    out_d[t * P:(t + 1) * P, :], ostg_box[0][:])
                        units.append(cp)
                return iter(units)

            # ----- startup: early DMA bandwidth is ~215GB/s TOTAL no matter
            # how many queues are used (they contend), so all inputs go on
            # ONE queue in exact first-use order. The VECTOR queue: the
            # hoisted ACT_TABLE_LOAD (1.3us) heads the scalar stream and
            # held the first data descriptor until 8.6us; the
            # GPSIMD queue has no table loads (DVE cannot issue DMAs).
            pu0 = proj_units(0, eng=nc.gpsimd)
            fill(pu0, 2)  # the two pair-0 weight-slice DMAs, on gpsimd
            xsw3 = xsw_d.rearrange("p (q k c) -> p q k c", q=SCH, k=8)
            # quarter 0 split in two k-halves: the first four contraction
            # matmuls gate on 512KB instead of 1MB (~2us earlier start)
            nc.gpsimd.dma_start(XT[:, 0, 0:4], xsw3[:, 0, 0:4])
            nc.gpsimd.dma_start(XT[:, 0, 4:8], xsw3[:, 0, 4:8])
            nc.gpsimd.dma_start(WV[:], wvsw_d.rearrange("p (k c) -> p k c", k=8))
            nc.gpsimd.dma_start(XT[:, 1], xsw3[:, 1])
            nc.gpsimd.dma_start(XT[:, 2], xsw3[:, 2])
            nc.gpsimd.dma_start(XT[:, 3], xsw3[:, 3])
            nc.gpsimd.dma_start(trimask[:], trimask_d[:, :])
            nc.gpsimd.dma_start(sel2[:], sel_d[:, :])

            norm_q = []
            stage_q = []
            # wsl+q0 -> sc0, +WV -> V group 0, +q1 -> sc1, V1, ...
            fill(pu0, 18)
            v_group(0)
            fill(pu0, 18)
            v_group(1)
            fill(pu0, 18)
            v_group(2)
            fill(pu0, 18)
            v_group(3)
            fill(pu0, 10 ** 6)
            def mk_filln(p):
                # pairs 0-2 have 74 filler units for 80 slots; place the
                # shortfall at c2's start (79% PE density) instead of the
                # pair's last slots (60% density -> HAM re-throttle)
                if p < NPAIR - 1:
                    return lambda c, j: 1 if (c == 2 and j < 6) else 2
                return lambda c, j: 2

            for p in range(NPAIR):
                if p < NPAIR - 1:
                    pu = proj_units(p + 1)
                    fillers = [pu, pu, pu, pu]
                else:
                    nc.sync.dma_start(
                        WO[:], wosw_d.rearrange("p (f d) -> p f d", f=NPAIR))
                    fillers = [iter(()), final_units(0), final_units(1),
                               final_units(2)]
                _attention_pair(nc, tc, psp, epool, stg, p, QT, KT, V, OT,
                                sel2, trimask, norm_q, stage_q, fillers,
                                mk_filln(p), True)
            # tail: pair-3 chunk-3 staging, then its norm split per t-block
            # and interleaved with the final projection of blocks 12..15
            stage_q.pop(0)()
            assert len(norm_q) == 1
            bc_box = []
            norm_bc, norm_mul_t = norm_q.pop(0)

            def pre(t):
                def fn(t=t):
                    if not bc_box:
                        bc_box.append(norm_bc(long_lived=True))
                    norm_mul_t(bc_box[0], t, rcp=True)
                return fn
            for fn in final_units(3, pre=pre, dma_eng=nc.scalar):
                fn()


def _attention_pair(nc, tc, psp, epool, stg, p, QT, KT, V, OT, sel2,
                    trimask, norm_q, stage_q, fillers, fill_n, drain_last):
    P = 128

    def fill(it, n):
        for _ in range(n):
            fn = next(it, None)
            if fn is None:
                return
            fn()

    last_pair = (p == NPAIR - 1)
    for c in range(SCH):
        filler = fillers[c]
        pend_stage = stage_q.pop(0) if stage_q else None
        pend_norm = norm_q.pop(0) if norm_q else None
        if last_pair:
            # pair 3's fillers (final projection) read OT at the first fill
            # slots, so staging+norm cannot be deferred into the j-loop
            if pend_stage is not None:
                pend_stage()
                pend_stage = None
            if pend_norm is not None:
                norm_bc, norm_mul_t = pend_norm
                norm_mul_t(norm_bc(), None)
                pend_norm = None
        o2 = psp.tile([P, 1024], F32, tag="o", bufs=1, name="o2")
        njt = 4 * c + 4
        pend_av = []
        for j in range(njt):
            d = j - 4 * c
            w = d * P if d >= 0 else 0
            s2 = psp.tile([P, 1024], F32, tag="s", bufs=2, name="s2")
            for hh in range(2):
                nc.tensor.matmul(
                    s2[:, hh * 512 + w: hh * 512 + 512],
                    KT[p][hh * 64:(hh + 1) * 64, j * P:(j + 1) * P],
                    QT[p][hh * 64:(hh + 1) * 64, c * 512 + w:(c + 1) * 512],
                    start=True, stop=True,
                )
            e2 = epool.tile([P, 1024], BF16, tag="e", name="e2")
            nc.scalar.activation(
                e2[:].rearrange("p (h q) -> p h q", h=2)[:, :, w:512],
                s2[:].rearrange("p (h q) -> p h q", h=2)[:, :, w:512],
                mybir.ActivationFunctionType.Exp,
                scale=float(SCALE),
            )
            if j == 0 and pend_stage is not None:
                # previous chunk's staging, emitted AFTER this chunk's first
                # exp: the stage copy waits on the last AVs, and putting it
                # first in the ACT queue would block the new chunk's exps
                # (head-of-line) and dip PE density enough to re-throttle
                # the HAM clock for ~10us
                pend_stage()
                pend_stage = None
            if j == (3 if c == 0 else 2) and pend_norm is not None:
                # chunk 0: pop one slot later -- the bc matmul's sums-hop
                # dependency (stage->sb2 DMA, ~2.3us after the chunk start)
                # races a j==2 pop (~2.2us in) at pair crossings
                norm_bc, norm_mul_t = pend_norm
                norm_mul_t(norm_bc(), None)
                pend_norm = None
            if d >= 0:
                # causal mask of the diagonal block, alternating WHOLE j's
                # between GpSimd and Vector: each j has its own e2 tile, so
                # this parallelizes the engines without the same-tile
                # cross-engine serialization a per-head split causes.
                eng = nc.gpsimd if (j % 2 == 0) else nc.vector
                for hh in range(2):
                    blk = e2[:, hh * 512 + w: hh * 512 + w + P]
                    eng.tensor_mul(blk, blk, trimask[:])
            fill(filler, fill_n(c, j))
            # chunk 0 is short and all-diagonal: defer AVs one slot deeper
            # so they clear the previous chunk's staging WAR and the mask
            # multiplies without stalling the PE
            if len(pend_av) == (3 if c == 0 else 2):
                pend_av.pop(0)()

            def av(j=j, w=w, e2=e2, o2=o2, njt=njt):
                for hh in range(2):
                    head = 2 * p + hh
                    nc.tensor.matmul(
                        o2[0:65, hh * 512 + w: hh * 512 + 512],
                        V[j][:, head * 65: head * 65 + 65],
                        e2[:, hh * 512 + w: hh * 512 + 512],
                        start=(j == 0), stop=(j == njt - 1),
                    )
            pend_av.append(av)
        for fn in pend_av:
            fn()
        # PSUM -> SBUF staging (engines cannot shift partitions; DMA cannot
        # read PSUM), then SBUF->SBUF DMAs to place head A / B rows. Row 0
        # of stage holds both heads' softmax denominators. Deferred into the
        # next chunk's j-loop (j==0) via stage_q.
        sb2_box = []

        def _staging(o2=o2, sb2_box=sb2_box, p=p, c=c):
            stage = stg.tile([65, 1024], BF16, tag="stage", bufs=2,
                             name="stage")
            # single whole-tile ACT copy (split variants measured worse),
            # sums in row 0, hop DMAs shift the body rows into OT
            nc.scalar.copy(stage[:], o2[0:65, :])
            nc.sync.dma_start(
                OT[p][0:64, c * 512:(c + 1) * 512], stage[1:65, 0:512])
            nc.sync.dma_start(
                OT[p][64:128, c * 512:(c + 1) * 512], stage[1:65, 512:1024])
            sb2 = stg.tile([2, 512], BF16, tag="sb2", bufs=3, name="sb2")
            nc.sync.dma_start(
                sb2[:, :], stage[0:1, :].rearrange("a (h c) -> a h c", h=2))
            sb2_box.append(sb2)
        stage_q.append(_staging)

        def _norm_bc(sb2_box=sb2_box, long_lived=False):
            sb2 = sb2_box[0]
            # Deferred into the next chunk's j-loop so the PE-stream
            # position of the bc matmul is past its dependencies.
            # Broadcast raw sums, then invert + multiply on DVE.
            # The tail norm (long_lived) must survive across final-unit
            # "pp" allocations, so it reuses the dead scores region; its
            # reciprocal runs on ACT (0.7us vs 3.4us on DVE) -- safe only
            # there because all exps are done, so no table-set thrash.
            if long_lived:
                bc = psp.tile([P, 1024], F32, tag="s", bufs=2,
                              name="s2")[:, 0:OC]
            else:
                bc = psp.tile([P, OC], F32, tag="pp", bufs=2, name="pp")
            # reciprocal lands in SBUF so the PSUM pp buffer is released
            # after ONE DVE op -- later filler matmuls reusing the pp tag
            # would otherwise stall on the whole norm chain
            rbc = stg.tile([P, OC], F32, tag="rbc", bufs=2, name="rbc")
            nc.tensor.matmul(bc[:], sel2[:], sb2[:], start=True, stop=True)
            if not long_lived:
                # approx_fast: ~18 correct bits (output path is bf16 anyway),
                # ~5x faster -- the 3.4us exact reciprocal blocked the DVE
                # FIFO at pair crossings, delaying the CAST that frees the
                # proj PSUM buffer (2.2us PE gap + HAM re-throttle)
                nc.vector.reciprocal_approx_fast(rbc[:], bc[:])
            return (bc, rbc)

        def _norm_mul_t(bcp, t, p=p, c=c, rcp=False):
            # t None: whole chunk; else one 128-col block of the chunk.
            # rcp: invert the block here (tail path: per-block reciprocal
            # pipelines with the per-t final projections instead of one
            # serial 3.4us reciprocal).
            bc, rbc = bcp
            if t is None:
                cols = slice(c * 512, (c + 1) * 512)
                bcols = slice(0, 512)
            else:
                tb = t - 4 * c
                cols = slice(c * 512 + tb * P, c * 512 + (tb + 1) * P)
                bcols = slice(tb * P, (tb + 1) * P)
            if rcp:
                nc.vector.reciprocal_approx_fast(rbc[:, bcols], bc[:, bcols])
            nc.vector.tensor_mul(
                OT[p][:, cols], OT[p][:, cols], rbc[:, bcols])
        norm_q.append((_norm_bc, _norm_mul_t))
        # drain leftover filler, but only when the next chunk doesn't
        # continue the same iterator (projection fillers span the pair;
        # pair 2 deliberately leaves 8 units for pair 3's chunk 0)
        if (c == SCH - 1 and drain_last) or \
                (c < SCH - 1 and fillers[c + 1] is not filler):
            fill(filler, 10 ** 6)


def _build():
    if "nc" in _CACHE:
        return _CACHE["nc"]
    nc = bacc.Bacc("TRN2", target_bir_lowering=False, debug=False)
    dram = {
        "xsw": nc.dram_tensor("xsw", [128, 16384], BF16,
                              kind="ExternalInput").ap(),
        "wqsw": nc.dram_tensor("wqsw", [512, 1024], BF16,
                               kind="ExternalInput").ap(),
        "wksw": nc.dram_tensor("wksw", [512, 1024], BF16,
                               kind="ExternalInput").ap(),
        "wvsw": nc.dram_tensor("wvsw", [128, 4096], BF16,
                               kind="ExternalInput").ap(),
        "wosw": nc.dram_tensor("wosw", [128, 4096], BF16,
                               kind="ExternalInput").ap(),
        "trimask": nc.dram_tensor("trimask", [128, 128], BF16,
                                  kind="ExternalInput").ap(),
        "sel2": nc.dram_tensor("sel2", [2, 128], BF16,
                               kind="ExternalInput").ap(),
        "out": nc.dram_tensor("out", [S, D], BF16, kind="ExternalOutput").ap(),
    }
    with tile.TileContext(nc) as tc:
        _emit(nc, tc, dram)
    nc.compile()
    _CACHE["nc"] = nc
    return nc


def make_in_maps(x, Wq, Wk, Wv, Wo):
    x = np.asarray(x, np.float32)
    Wq = np.asarray(Wq, np.float32)
    Wk = np.asarray(Wk, np.float32)
    Wv = np.asarray(Wv, np.float32)
    Wo = np.asarray(Wo, np.float32)
    tri = np.tril(np.ones((128, 128), np.float32)).T.astype(NPBF16)
    sel = np.zeros((2, 128), np.float32)
    sel[0, 0:64] = 1.0
    sel[1, 64:128] = 1.0
    in_maps = []
    for cidx in range(8):
        b, g = divmod(cidx, 2)
        sl = slice(g * OC, (g + 1) * OC)
        xT = np.ascontiguousarray(x[b].T)                   # [1024, 2048]
        # [p, q, k, c]: xT[k*128+p, q*512+c]
        xsw = np.ascontiguousarray(
            xT.reshape(8, 128, 4, 512).transpose(1, 2, 0, 3)
        ).reshape(128, 16384)
        WqT = Wq[sl, :].T                                   # [1024, 512]
        WkT = Wk[sl, :].T
        WvT = Wv[sl, :].T
        WoT = Wo[:, sl].T                                   # [512, 1024]
        # wsl layout per pair: [pair*128+p, k*128+c] = WqT[k*128+p, pair*128+c]
        wqsw = np.ascontiguousarray(
            WqT.reshape(8, 128, 4, 128).transpose(2, 1, 0, 3)
        ).reshape(512, 1024)
        wksw = np.ascontiguousarray(
            WkT.reshape(8, 128, 4, 128).transpose(2, 1, 0, 3)
        ).reshape(512, 1024)
        # [p, k*512+c] = WvT[k*128+p, c]
        wvsw = np.ascontiguousarray(
            WvT.reshape(8, 128, 512).transpose(1, 0, 2)
        ).reshape(128, 4096)
        # [p, p4*1024+d] = WoT[p4*128+p, d]
        wosw = np.ascontiguousarray(
            WoT.reshape(4, 128, 1024).transpose(1, 0, 2)
        ).reshape(128, 4096)
        in_maps.append({
            "xsw": xsw.astype(NPBF16),
            "wqsw": wqsw.astype(NPBF16),
            "wksw": wksw.astype(NPBF16),
            "wvsw": wvsw.astype(NPBF16),
            "wosw": wosw.astype(NPBF16),
            "trimask": tri,
            "sel2": sel.astype(NPBF16),
        })
    return in_maps


def combine(results):
    parts = [np.asarray(results[c]["out"]).astype(np.float32)
             for c in range(8)]
    return np.stack([parts[2 * b] + parts[2 * b + 1] for b in range(B)])


def kernel(**inputs):
    nc = _build()
    in_maps = make_in_maps(inputs["x"], inputs["Wq"], inputs["Wk"],
                           inputs["Wv"], inputs["Wo"])
    res = bass_utils.run_bass_kernel_spmd(nc, in_maps, core_ids=list(range(8)))
    return combine(res.results)


def run_traced(**inputs):
    nc = _build()
    in_maps = make_in_maps(inputs["x"], inputs["Wq"], inputs["Wk"],
                           inputs["Wv"], inputs["Wo"])
    res = bass_utils.run_bass_kernel_spmd(
        nc, in_maps, core_ids=list(range(8)), trace=True)
    return combine(res.results), res


def run_timed(**inputs):
    """One more traced execution, returning just the exec time (repeat
    measurements to average out the chip power-state noise)."""
    nc = _build()
    in_maps = make_in_maps(inputs["x"], inputs["Wq"], inputs["Wk"],
                           inputs["Wv"], inputs["Wo"])
    res = bass_utils.run_bass_kernel_spmd(
        nc, in_maps, core_ids=list(range(8)), trace=True)
    return res.exec_time_ns



# revision 9
# speedup vs baseline: 1.1200x; 1.0493x over previous
"""Causal multi-head self-attention on 8 TRN2 NeuronCores.

Sharding: batch (4) x head-group (2) -> 8 cores. Each core computes, for its
batch b and its 8 heads, the attention output projected through its slice of
Wo; the host sums the two partial outputs per batch.

Per-core layout (P = 128 partitions):
  XT   [128, 4, 8, 512] bf16 - x[b].T swizzled (quarter, d-chunk, col),
                               host pre-swizzled so each input quarter is one
                               contiguous 1MB DMA (descriptor-issue bound
                               startup: ~650ns per DMA instruction on the
                               HWDGE queues).
  WV   [128, 8, 512]  bf16 - Wv slice, host-swizzled, one DMA
  wsl  [128, 1024]    bf16 - per-pair Wq/Wk slices, host-swizzled, one DMA
  WO   [128, 4, 1024] f32r - Wo slice, host-swizzled, one DMA
  QT/KT [128, 2048] bf16 - head-dim on partitions (pair p -> tile p)
  V    16 tiles [128, 520] bf16 - seq on partitions, per-head 65-col groups
                                  (64 V cols + a ones col for row sums)
  scores computed transposed: S.T[k, q] = K @ Q.T, per head-pair via
  row-group packing (head A rows 0-63, head B rows 64-127).
  exp on ACT with fused 1/sqrt(dk) scale; causal via window-trimmed matmuls
  plus one [128,128] lower-triangle mask multiply on GpSimd (its queue is
  empty, keeping the exp->mask->AV chain off the vector-engine backlog).
  AV: O.T[65, q] += V_aug.T @ E.T accumulated over k-tiles in PSUM
  (row 0 = softmax denominators, ones col first). Normalize via vector
  reciprocal + PE broadcast (ones selector matmul). Final projection in
  bf16 (f32r would lower to the 2-pass fp32_mode=HIGH matmul path).
  Input DMAs are issued from both HWDGE queues (sync + scalar) to halve
  descriptor-issue latency at startup.
"""

import numpy as np
import ml_dtypes

import concourse.bass as bass
import concourse.tile as tile
from concourse import bacc, mybir
from concourse import bass_utils

F32 = mybir.dt.float32
F32R = mybir.dt.float32r
BF16 = mybir.dt.bfloat16
NPBF16 = ml_dtypes.bfloat16

B, S, D, H, DK = 4, 2048, 1024, 16, 64
HC = 8          # heads per core
NPAIR = 4       # head pairs per core
OC = 512        # output dims per core (= HC * DK)
KT_N = 16       # seq k-tiles of 128
SCH = 4         # seq chunks of 512
SCALE = 1.0 / np.sqrt(np.float32(DK))

_CACHE = {}


def _emit(nc, tc, dram):
    P = 128
    xsw_d, wqsw_d, wksw_d, wvsw_d, wosw_d, trimask_d, sel_d, out_d = (
        dram["xsw"], dram["wqsw"], dram["wksw"], dram["wvsw"], dram["wosw"],
        dram["trimask"], dram["sel2"], dram["out"],
    )

    import contextlib
    ctx = contextlib.ExitStack()
    with ctx:
        # ---------------- persistent SBUF ----------------
        per = ctx.enter_context(tc.tile_pool(name="per", bufs=1))
        trimask = per.tile([P, P], BF16, tag="trimask", name="trimask")
        sel2 = per.tile([2, P], BF16, tag="sel2", name="sel2")

        QT = [per.tile([P, S], BF16, tag=f"QT{p}", name=f"QT{p}") for p in range(NPAIR)]
        KT = [per.tile([P, S], BF16, tag=f"KT{p}", name=f"KT{p}") for p in range(NPAIR)]
        V = [per.tile([P, HC * 65], BF16, tag=f"V{t}", name=f"V{t}") for t in range(KT_N)]
        # OT / WO / stage are bf16: f32r operands lower to the 2-pass
        # fp32_mode=HIGH matmul path (~3x slower, LDWEIGHTS not overlapped)
        OT = [per.tile([P, S], BF16, tag=f"OT{p}", name=f"OT{p}") for p in range(NPAIR)]
        WO = per.tile([P, NPAIR, D], BF16, tag="WO", name="WO")

        epool = ctx.enter_context(tc.tile_pool(name="epool", bufs=6))
        stg = ctx.enter_context(tc.tile_pool(name="stg", bufs=1))

        with tc.tile_pool(name="proj_in", bufs=1) as proj_in, \
             tc.tile_pool(name="wqk", bufs=2) as wqk_pool, \
             tc.tile_pool(name="ps", bufs=1, space="PSUM") as psp:
            XT = proj_in.tile([P, SCH, 8, 512], BF16, tag="XT", name="XT")
            WV = proj_in.tile([P, 8, 512], BF16, tag="WV", name="WV")

            # PSUM budget (8 banks of [128,512]f32):
            #   pp: 2 x [128,512]  = 2 banks (projection accumulators)
            #   s:  2 x [128,1024] = 4 banks (scores, both heads)
            #   o:  1 x [128,1024] = 2 banks (output accum, both heads)
            def pp_tile():
                return psp.tile([P, OC], F32, tag="pp", bufs=2, name="pp")

            def fill(it, n):
                for _ in range(n):
                    fn = next(it, None)
                    if fn is None:
                        return
                    fn()

            # ----- V projection: V[st] = x[st*128:+128, :] @ WvT, seq on
            # partitions. Emitted in quarter groups so it can interleave with
            # pair-0 QK projections while input quarters are still landing.
            def v_group(g):
                for st in range(4 * g, 4 * g + 4):
                    qq, bb = divmod(st, 4)
                    ps = pp_tile()
                    for k in range(8):
                        nc.tensor.matmul(
                            ps[:], XT[:, qq, k, bb * P:(bb + 1) * P], WV[:, k],
                            start=(k == 0), stop=(k == 7),
                        )
                    # ones column FIRST: the AV sums row lands on o2 row 0
                    v3 = V[st].rearrange("p (h d) -> p h d", d=65)
                    nc.vector.tensor_copy(
                        v3[:, :, 1:65], ps[:].rearrange("p (h d) -> p h d", d=64)
                    )
                    nc.gpsimd.memset(v3[:, :, 0:1], 1.0)

            # ----- QK projections + attention, pair by pair.
            # The PE executes its stream in order and the attention j-loop is
            # ACT(exp)-bound, so projection / final-projection matmuls are
            # interleaved as per-MM filler units inside the j-loop.
            def proj_units(p, eng=None):
                # yields: 2 weight-load units, then per sc the q-proj and
                # k-proj of that chunk (so startup can gate on XT quarters)
                units = []
                load_units = []
                boxes = []
                for (wd,) in ((wqsw_d,), (wksw_d,)):
                    def load_w(wd=wd):
                        wsl = wqk_pool.tile([P, 8 * P], BF16, tag="wsl",
                                            name="wsl")
                        (eng or nc.sync).dma_start(
                            wsl[:], wd[p * P:(p + 1) * P, :])
                        return wsl
                    wsl_box = []
                    boxes.append(wsl_box)
                    load_units.append(lambda wsl_box=wsl_box, load_w=load_w:
                                      wsl_box.append(load_w()))
                for sc in range(SCH):
                    for wi, dst in ((0, QT[p]), (1, KT[p])):
                        ps_box = []
                        wsl_box = boxes[wi]
                        for k in range(8):
                            def mm(k=k, sc=sc, ps_box=ps_box,
                                   wsl_box=wsl_box):
                                if k == 0:
                                    ps_box.append(pp_tile())
                                nc.tensor.matmul(
                                    ps_box[0][:],
                                    wsl_box[0][:, k * P:(k + 1) * P],
                                    XT[:, sc, k],
                                    start=(k == 0), stop=(k == 7),
                                )
                            units.append(mm)
                        def cp(sc=sc, ps_box=ps_box, dst=dst):
                            nc.vector.tensor_copy(
                                dst[:, sc * 512:(sc + 1) * 512], ps_box[0][:])
                        units.append(cp)
                # both weight-slice DMAs lead the unit stream so neither
                # projection's first matmul waits on its load
                return iter(load_units + units)

            def final_units(cc, pre=None, dma_eng=None):
                # pre: optional per-t hook run before t's matmuls (tail norm)
                # dma_eng: queue for the output DMA (tail uses scalar: ACT is
                # idle there and same-queue issue skips a semaphore hop)
                units = []
                for t in range(4 * cc, 4 * cc + 4):
                    if pre is not None:
                        units.append(pre(t))
                    ostg_box = []
                    for oc in range(2):
                        ps_box = []
                        for p4 in range(NPAIR):
                            def mm(p4=p4, t=t, oc=oc, ps_box=ps_box):
                                if p4 == 0:
                                    ps_box.append(
                                        psp.tile([P, OC], F32, tag="pp",
                                                 bufs=2, name="pp"))
                                nc.tensor.matmul(
                                    ps_box[0][:],
                                    OT[p4][:, t * P:(t + 1) * P],
                                    WO[:, p4, oc * 512:(oc + 1) * 512],
                                    start=(p4 == 0), stop=(p4 == NPAIR - 1),
                                )
                            units.append(mm)
                        def cp(t=t, oc=oc, ps_box=ps_box, ostg_box=ostg_box):
                            if oc == 0:
                                ostg_box.append(
                                    stg.tile([P, 2 * OC], BF16, tag="ostg",
                                             bufs=3, name="ostg"))
                            # as a pair-3 attention filler (dma_eng None)
                            # this copy must NOT sit on ACT: each one pushes
                            # the exp chain (the attention pacer) back
                            # ~0.7us. DVE is light there (no proj CASTs).
                            cpeng = nc.scalar if dma_eng is not None \
                                else nc.vector
                            if cpeng is nc.scalar:
                                nc.scalar.copy(
                                    ostg_box[0][:, oc * 512:(oc + 1) * 512],
                                    ps_box[0][:])
                            else:
                                nc.vector.tensor_copy(
                                    ostg_box[0][:, oc * 512:(oc + 1) * 512],
                                    ps_box[0][:])
                            if dma_eng is not None:
                                # tail: DMA each half right after its copy
                                # so transfer overlaps the other half's copy
                                dma_eng.dma_start(
                                    out_d[t * P:(t + 1) * P,
                                          oc * 512:(oc + 1) * 512],
                                    ostg_box[0][:, oc * 512:(oc + 1) * 512])
                            elif oc == 1:
                                nc.sync.dma_start(
                                    out_d[t * P:(t + 1) * P, :], ostg_box[0][:])
                        units.append(cp)
                return iter(units)

            # ----- startup: early DMA bandwidth is ~215GB/s TOTAL no matter
            # how many queues are used (they contend), so all inputs go on
            # ONE queue in exact first-use order. The VECTOR queue: the
            # hoisted ACT_TABLE_LOAD (1.3us) heads the scalar stream and
            # held the first data descriptor until 8.6us; the
            # GPSIMD queue has no table loads (DVE cannot issue DMAs).
            pu0 = proj_units(0, eng=nc.gpsimd)
            fill(pu0, 1)  # pair-0 Wq-slice DMA leads the queue
            xsw3 = xsw_d.rearrange("p (q k c) -> p q k c", q=SCH, k=8)
            # early transfers are serial on the queue at ~130GB/s: quarter 0
            # in k-quarters, wslk after the first two so the first matmuls
            # gate on 512KB of cumulative traffic instead of 1MB
            nc.gpsimd.dma_start(XT[:, 0, 0:2], xsw3[:, 0, 0:2])
            nc.gpsimd.dma_start(XT[:, 0, 2:4], xsw3[:, 0, 2:4])
            fill(pu0, 1)  # Wk-slice DMA
            nc.gpsimd.dma_start(XT[:, 0, 4:8], xsw3[:, 0, 4:8])
            nc.gpsimd.dma_start(WV[:], wvsw_d.rearrange("p (k c) -> p k c", k=8))
            nc.gpsimd.dma_start(XT[:, 1], xsw3[:, 1])
            nc.gpsimd.dma_start(XT[:, 2], xsw3[:, 2])
            nc.gpsimd.dma_start(XT[:, 3], xsw3[:, 3])
            nc.gpsimd.dma_start(trimask[:], trimask_d[:, :])
            nc.gpsimd.dma_start(sel2[:], sel_d[:, :])

            norm_q = []
            stage_q = []
            # wsl+q0 -> sc0, +WV -> V group 0, +q1 -> sc1, V1, ...
            fill(pu0, 18)
            v_group(0)
            fill(pu0, 18)
            v_group(1)
            fill(pu0, 18)
            v_group(2)
            fill(pu0, 18)
            v_group(3)
            fill(pu0, 10 ** 6)
            def mk_filln(p):
                # pairs 0-2 have 74 filler units for 80 slots; place the
                # shortfall at c2's start (79% PE density) instead of the
                # pair's last slots (60% density -> HAM re-throttle)
                if p < NPAIR - 1:
                    return lambda c, j: 1 if (c == 2 and j < 6) else 2
                return lambda c, j: 0 if j < 4 else 3

            for p in range(NPAIR):
                if p < NPAIR - 1:
                    pu = proj_units(p + 1)
                    fillers = [pu, pu, pu, pu]
                else:
                    nc.sync.dma_start(
                        WO[:], wosw_d.rearrange("p (f d) -> p f d", f=NPAIR))
                    fillers = [iter(()), final_units(0), final_units(1),
                               final_units(2)]
                _attention_pair(nc, tc, psp, epool, stg, p, QT, KT, V, OT,
                                sel2, trimask, norm_q, stage_q, fillers,
                                mk_filln(p), True)
            # tail: pair-3 chunk-3 staging, then its norm split per t-block
            # and interleaved with the final projection of blocks 12..15
            stage_q.pop(0)()
            assert len(norm_q) == 1
            bc_box = []
            norm_bc, norm_mul_t = norm_q.pop(0)

            def pre(t):
                def fn(t=t):
                    if not bc_box:
                        bc_box.append(norm_bc(long_lived=True))
                    norm_mul_t(bc_box[0], t, rcp=True)
                return fn
            for fn in final_units(3, pre=pre, dma_eng=nc.scalar):
                fn()


def _attention_pair(nc, tc, psp, epool, stg, p, QT, KT, V, OT, sel2,
                    trimask, norm_q, stage_q, fillers, fill_n, drain_last):
    P = 128

    def fill(it, n):
        for _ in range(n):
            fn = next(it, None)
            if fn is None:
                return
            fn()

    last_pair = (p == NPAIR - 1)
    for c in range(SCH):
        filler = fillers[c]
        pend_stage = stage_q.pop(0) if stage_q else None
        pend_norm = norm_q.pop(0) if norm_q else None
        # pair 3's staging+norm defer into the j-loop like other pairs;
        # its final-projection fillers are held until j>=4 (fill_n 0/3) so
        # the first OT reads land after the norm chain completes -- running
        # staging+norm at chunk start stalled the first filler ~2.1us
        o2 = psp.tile([P, 1024], F32, tag="o", bufs=1, name="o2")
        njt = 4 * c + 4
        pend_av = []
        for j in range(njt):
            d = j - 4 * c
            w = d * P if d >= 0 else 0
            s2 = psp.tile([P, 1024], F32, tag="s", bufs=2, name="s2")
            for hh in range(2):
                nc.tensor.matmul(
                    s2[:, hh * 512 + w: hh * 512 + 512],
                    KT[p][hh * 64:(hh + 1) * 64, j * P:(j + 1) * P],
                    QT[p][hh * 64:(hh + 1) * 64, c * 512 + w:(c + 1) * 512],
                    start=True, stop=True,
                )
            e2 = epool.tile([P, 1024], BF16, tag="e", name="e2")
            nc.scalar.activation(
                e2[:].rearrange("p (h q) -> p h q", h=2)[:, :, w:512],
                s2[:].rearrange("p (h q) -> p h q", h=2)[:, :, w:512],
                mybir.ActivationFunctionType.Exp,
                scale=float(SCALE),
            )
            if j == 0 and pend_stage is not None:
                # previous chunk's staging, emitted AFTER this chunk's first
                # exp: the stage copy waits on the last AVs, and putting it
                # first in the ACT queue would block the new chunk's exps
                # (head-of-line) and dip PE density enough to re-throttle
                # the HAM clock for ~10us
                pend_stage()
                pend_stage = None
            if j == (3 if c == 0 else 2) and pend_norm is not None:
                # chunk 0: pop one slot later -- the bc matmul's sums-hop
                # dependency (stage->sb2 DMA, ~2.3us after the chunk start)
                # races a j==2 pop (~2.2us in) at pair crossings
                norm_bc, norm_mul_t = pend_norm
                norm_mul_t(norm_bc(), None)
                pend_norm = None
            if d >= 0:
                # causal mask of the diagonal block, alternating WHOLE j's
                # between GpSimd and Vector: each j has its own e2 tile, so
                # this parallelizes the engines without the same-tile
                # cross-engine serialization a per-head split causes.
                eng = nc.gpsimd if (j % 2 == 0) else nc.vector
                for hh in range(2):
                    blk = e2[:, hh * 512 + w: hh * 512 + w + P]
                    eng.tensor_mul(blk, blk, trimask[:])
            fill(filler, fill_n(c, j))
            # chunk 0 is short and all-diagonal: defer AVs one slot deeper
            # so they clear the previous chunk's staging WAR and the mask
            # multiplies without stalling the PE
            if len(pend_av) == (3 if c == 0 else 2):
                pend_av.pop(0)()

            def av(j=j, w=w, e2=e2, o2=o2, njt=njt):
                for hh in range(2):
                    head = 2 * p + hh
                    nc.tensor.matmul(
                        o2[0:65, hh * 512 + w: hh * 512 + 512],
                        V[j][:, head * 65: head * 65 + 65],
                        e2[:, hh * 512 + w: hh * 512 + 512],
                        start=(j == 0), stop=(j == njt - 1),
                    )
            pend_av.append(av)
        for fn in pend_av:
            fn()
        # PSUM -> SBUF staging (engines cannot shift partitions; DMA cannot
        # read PSUM), then SBUF->SBUF DMAs to place head A / B rows. Row 0
        # of stage holds both heads' softmax denominators. Deferred into the
        # next chunk's j-loop (j==0) via stage_q.
        sb2_box = []

        def _staging(o2=o2, sb2_box=sb2_box, p=p, c=c):
            stage = stg.tile([65, 1024], BF16, tag="stage", bufs=2,
                             name="stage")
            # two half-tile DVE copies (ACT is the j-loop's busiest engine:
            # a 1.07us ACT copy here pushed every exp of the chunk back and
            # surfaced as s2-WAR PE gaps at chunk boundaries; DVE has slack),
            # sums in row 0, hop DMAs shift the body rows into OT
            nc.vector.tensor_copy(stage[:, 0:512], o2[0:65, 0:512])
            nc.sync.dma_start(
                OT[p][0:64, c * 512:(c + 1) * 512], stage[1:65, 0:512])
            nc.vector.tensor_copy(stage[:, 512:1024], o2[0:65, 512:1024])
            nc.sync.dma_start(
                OT[p][64:128, c * 512:(c + 1) * 512], stage[1:65, 512:1024])
            sb2 = stg.tile([2, 512], BF16, tag="sb2", bufs=3, name="sb2")
            nc.sync.dma_start(
                sb2[:, :], stage[0:1, :].rearrange("a (h c) -> a h c", h=2))
            sb2_box.append(sb2)
        stage_q.append(_staging)

        def _norm_bc(sb2_box=sb2_box, long_lived=False):
            sb2 = sb2_box[0]
            # Deferred into the next chunk's j-loop so the PE-stream
            # position of the bc matmul is past its dependencies.
            # Broadcast raw sums, then invert + multiply on DVE.
            # The tail norm (long_lived) must survive across final-unit
            # "pp" allocations, so it reuses the dead scores region; its
            # reciprocal runs on ACT (0.7us vs 3.4us on DVE) -- safe only
            # there because all exps are done, so no table-set thrash.
            if long_lived:
                bc = psp.tile([P, 1024], F32, tag="s", bufs=2,
                              name="s2")[:, 0:OC]
            else:
                bc = psp.tile([P, OC], F32, tag="pp", bufs=2, name="pp")
            # reciprocal lands in SBUF so the PSUM pp buffer is released
            # after ONE DVE op -- later filler matmuls reusing the pp tag
            # would otherwise stall on the whole norm chain
            rbc = stg.tile([P, OC], F32, tag="rbc", bufs=2, name="rbc")
            nc.tensor.matmul(bc[:], sel2[:], sb2[:], start=True, stop=True)
            if not long_lived:
                # approx_fast: ~18 correct bits (output path is bf16 anyway),
                # ~5x faster -- the 3.4us exact reciprocal blocked the DVE
                # FIFO at pair crossings, delaying the CAST that frees the
                # proj PSUM buffer (2.2us PE gap + HAM re-throttle)
                nc.vector.reciprocal_approx_fast(rbc[:], bc[:])
            return (bc, rbc)

        def _norm_mul_t(bcp, t, p=p, c=c, rcp=False):
            # t None: whole chunk; else one 128-col block of the chunk.
            # rcp: invert the block here (tail path: per-block reciprocal
            # pipelines with the per-t final projections instead of one
            # serial 3.4us reciprocal).
            bc, rbc = bcp
            if t is None:
                cols = slice(c * 512, (c + 1) * 512)
                bcols = slice(0, 512)
            else:
                tb = t - 4 * c
                cols = slice(c * 512 + tb * P, c * 512 + (tb + 1) * P)
                bcols = slice(tb * P, (tb + 1) * P)
            if rcp:
                nc.vector.reciprocal_approx_fast(rbc[:, bcols], bc[:, bcols])
            nc.vector.tensor_mul(
                OT[p][:, cols], OT[p][:, cols], rbc[:, bcols])
        norm_q.append((_norm_bc, _norm_mul_t))
        # drain leftover filler, but only when the next chunk doesn't
        # continue the same iterator (projection fillers span the pair;
        # pair 2 deliberately leaves 8 units for pair 3's chunk 0)
        if (c == SCH - 1 and drain_last) or \
                (c < SCH - 1 and fillers[c + 1] is not filler):
            fill(filler, 10 ** 6)


def _build():
    if "nc" in _CACHE:
        return _CACHE["nc"]
    nc = bacc.Bacc("TRN2", target_bir_lowering=False, debug=False)
    dram = {
        "xsw": nc.dram_tensor("xsw", [128, 16384], BF16,
                              kind="ExternalInput").ap(),
        "wqsw": nc.dram_tensor("wqsw", [512, 1024], BF16,
                               kind="ExternalInput").ap(),
        "wksw": nc.dram_tensor("wksw", [512, 1024], BF16,
                               kind="ExternalInput").ap(),
        "wvsw": nc.dram_tensor("wvsw", [128, 4096], BF16,
                               kind="ExternalInput").ap(),
        "wosw": nc.dram_tensor("wosw", [128, 4096], BF16,
                               kind="ExternalInput").ap(),
        "trimask": nc.dram_tensor("trimask", [128, 128], BF16,
                                  kind="ExternalInput").ap(),
        "sel2": nc.dram_tensor("sel2", [2, 128], BF16,
                               kind="ExternalInput").ap(),
        "out": nc.dram_tensor("out", [S, D], BF16, kind="ExternalOutput").ap(),
    }
    with tile.TileContext(nc) as tc:
        _emit(nc, tc, dram)
    nc.compile()
    _CACHE["nc"] = nc
    return nc


def make_in_maps(x, Wq, Wk, Wv, Wo):
    x = np.asarray(x, np.float32)
    Wq = np.asarray(Wq, np.float32)
    Wk = np.asarray(Wk, np.float32)
    Wv = np.asarray(Wv, np.float32)
    Wo = np.asarray(Wo, np.float32)
    tri = np.tril(np.ones((128, 128), np.float32)).T.astype(NPBF16)
    sel = np.zeros((2, 128), np.float32)
    sel[0, 0:64] = 1.0
    sel[1, 64:128] = 1.0
    in_maps = []
    for cidx in range(8):
        b, g = divmod(cidx, 2)
        sl = slice(g * OC, (g + 1) * OC)
        xT = np.ascontiguousarray(x[b].T)                   # [1024, 2048]
        # [p, q, k, c]: xT[k*128+p, q*512+c]
        xsw = np.ascontiguousarray(
            xT.reshape(8, 128, 4, 512).transpose(1, 2, 0, 3)
        ).reshape(128, 16384)
        WqT = Wq[sl, :].T                                   # [1024, 512]
        WkT = Wk[sl, :].T
        WvT = Wv[sl, :].T
        WoT = Wo[:, sl].T                                   # [512, 1024]
        # wsl layout per pair: [pair*128+p, k*128+c] = WqT[k*128+p, pair*128+c]
        wqsw = np.ascontiguousarray(
            WqT.reshape(8, 128, 4, 128).transpose(2, 1, 0, 3)
        ).reshape(512, 1024)
        wksw = np.ascontiguousarray(
            WkT.reshape(8, 128, 4, 128).transpose(2, 1, 0, 3)
        ).reshape(512, 1024)
        # [p, k*512+c] = WvT[k*128+p, c]
        wvsw = np.ascontiguousarray(
            WvT.reshape(8, 128, 512).transpose(1, 0, 2)
        ).reshape(128, 4096)
        # [p, p4*1024+d] = WoT[p4*128+p, d]
        wosw = np.ascontiguousarray(
            WoT.reshape(4, 128, 1024).transpose(1, 0, 2)
        ).reshape(128, 4096)
        in_maps.append({
            "xsw": xsw.astype(NPBF16),
            "wqsw": wqsw.astype(NPBF16),
            "wksw": wksw.astype(NPBF16),
            "wvsw": wvsw.astype(NPBF16),
            "wosw": wosw.astype(NPBF16),
            "trimask": tri,
            "sel2": sel.astype(NPBF16),
        })
    return in_maps


def combine(results):
    parts = [np.asarray(results[c]["out"]).astype(np.float32)
             for c in range(8)]
    return np.stack([parts[2 * b] + parts[2 * b + 1] for b in range(B)])


def kernel(**inputs):
    nc = _build()
    in_maps = make_in_maps(inputs["x"], inputs["Wq"], inputs["Wk"],
                           inputs["Wv"], inputs["Wo"])
    res = bass_utils.run_bass_kernel_spmd(nc, in_maps, core_ids=list(range(8)))
    return combine(res.results)


def run_traced(**inputs):
    nc = _build()
    in_maps = make_in_maps(inputs["x"], inputs["Wq"], inputs["Wk"],
                           inputs["Wv"], inputs["Wo"])
    res = bass_utils.run_bass_kernel_spmd(
        nc, in_maps, core_ids=list(range(8)), trace=True)
    return combine(res.results), res


def run_timed(**inputs):
    """One more traced execution, returning just the exec time (repeat
    measurements to average out the chip power-state noise)."""
    nc = _build()
    in_maps = make_in_maps(inputs["x"], inputs["Wq"], inputs["Wk"],
                           inputs["Wv"], inputs["Wo"])
    res = bass_utils.run_bass_kernel_spmd(
        nc, in_maps, core_ids=list(range(8)), trace=True)
    return res.exec_time_ns

